# revision 8
# baseline (speedup 1.0000x reference)
"""GAT (2-layer, 4-head) on 8 Trainium2 NeuronCores.

Strategy (dst-sharded, per sharding hint):
  - Nodes partitioned into 8 contiguous blocks of 6250; core c owns block c
    and all edges whose dst lies in it. Per core, dst nodes map to 200
    windows x 32 lanes with per-lane chunk capacities (3 lo + 2 hi chunks of
    128 edge slots per window); edge softmax runs on a compact
    [128, 40, H] tensor and a one-hot [slots, (h,lane)] lhsT drives the
    PE segment-sum matmuls.
  - Table rows carry only R = feat @ T (256 bf16 = 512B/row), where T is
    invertible, block-diagonal per head, with block-column 0 equal to al_h:
    column 64h of R IS el_h, so el needs no extra row space and rows shrank
    768B -> 512B (33% less AllGather + gather traffic). True features are
    recovered tile-level via M @ Tinv (block-diag, 2 matmuls of N=128) before
    the elu, fused into the next phase-A / output stage.
  - Softmax denominators: per chunk one small matmul (lhsT = lane one-hot
    mask, rhs = compact exp sv) accumulates denom [32 lanes, H] per window in
    node-major partition order; [128, 4] denominator tiles stay in SBUF and
    are divided out at the consumer tile stage (no "ones" column in rows).
  - Gathers are software-pipelined two groups ahead of their consumers;
    dead-slot masking is folded into the el extraction add.
  - Final output is produced in transposed [d, node] layout; the host
    transposes back during unpermute.
"""

import sys

sys.path.insert(0, "/opt/trn_rl_repo")

import numpy as np

import concourse.bacc as bacc
import concourse.mybir as mybir
import concourse.tile as tile
from concourse import bass
from concourse.bass_utils import run_bass_kernel_spmd
from concourse.masks import make_identity

# ---------------- problem constants ----------------
N = 50000
E = 800000
D = 256           # in feats = H*F
H = 4
F = 64
NC = 8
NLOC = N // NC    # 6250
NEG_SLOPE = 0.2

# ---------------- sharding constants ----------------
WSPAN = 32        # dsts (lanes) per window
NWIN = 200        # windows per core
C_LO = 3          # lo chunks per window
C_HI = 2          # hi chunks per window
C_W = C_LO + C_HI
WG = 8            # windows per group
NGRP = NWIN // WG          # 25
CH_PER_G = WG * C_W        # 40 chunks per group
CALLS_PER_G = CH_PER_G // 8  # 5 gather calls per group (1024 idx each)
NCH = NWIN * C_W           # 1000 chunks per layer
SLOTS = NCH * 128          # 128000 edge slots
NPAD = NWIN * WSPAN        # 6400 padded rows per core
RW = 256          # table row width (bf16): R = feat @ T only (512B rows)
WGT = D + H       # fused weight cols: 256 (W@T) + 4 er
LO_ROWS = 5 * NPAD          # 32000 (int16-safe)
NTILES = NPAD // 128        # 50
NEG_BIG = -1.0e30

_PROGRAM_CACHE = {}


# =====================================================================
# host-side graph preprocessing
# =====================================================================

def _design_thresholds(nlo, nhi):
    """Descending per-lane total-capacity thresholds L (lo, sum<=384) and
    Hh (hi, sum<=256) making the joint lane assignment feasible."""
    L = np.sort(nlo)[::-1][::NWIN][:WSPAN].astype(np.int64)
    Hh = np.sort(nhi)[::-1][::NWIN][:WSPAN].astype(np.int64)
    BUD_L, BUD_H = C_LO * 128, C_HI * 128
    for _ in range(8000):
        B_lo = np.searchsorted(-L, -nlo, side="right") - 1
        B_hi = np.searchsorted(-Hh, -nhi, side="right") - 1
        B = np.minimum(B_lo, B_hi)
        assert (B >= 0).all(), "dst exceeds top lane capacity"
        cum = np.cumsum(np.bincount(B, minlength=WSPAN))
        viol = cum - NWIN * np.arange(1, WSPAN + 1)
        if (viol[:-1] <= 0).all():
            assert cum[-1] <= NWIN * WSPAN
            return L, Hh, B
        k = int(np.flatnonzero(viol[:-1] > 0)[0])
        blocked = B <= k
        gain_L = gain_H = -1
        if L.sum() < BUD_L and L[k + 1] < L[k]:
            gain_L = int((blocked & (nlo == L[k + 1] + 1) & (nhi <= Hh[k + 1])).sum())
        if Hh.sum() < BUD_H and Hh[k + 1] < Hh[k]:
            gain_H = int((blocked & (nhi == Hh[k + 1] + 1) & (nlo <= L[k + 1])).sum())
        assert gain_L >= 0 or gain_H >= 0, "lane threshold design infeasible"
        if gain_L >= gain_H:
            L[k + 1] += 1
        else:
            Hh[k + 1] += 1
    raise RuntimeError("threshold design did not converge")


def _split_caps(L, n_chunks):
    """Split per-lane totals L into n_chunks per-chunk caps with each chunk's
    column sum exactly 128. Extra capacity (128*n - sum(L)) goes to lane 0."""
    caps = np.zeros((n_chunks, WSPAN), np.int64)
    room = np.full(n_chunks, 128, np.int64)
    for j in np.argsort(-L, kind="stable"):
        left = int(L[j])
        while left > 0:
            order = np.argsort(-room, kind="stable")
            share = -(-left // len(order))
            for c in order:
                take = min(share, left, room[c])
                caps[c, j] += take
                room[c] -= take
                left -= take
                if left == 0:
                    break
    caps[:, 0] += room  # leftover slots become extra lane-0 capacity
    assert (caps.sum(axis=1) == 128).all()
    return caps


def _rank_within_group(keys):
    """For each element, its 0-based rank among equal keys (stable)."""
    order = np.argsort(keys, kind="stable")
    sk = keys[order]
    starts = np.r_[0, np.flatnonzero(sk[1:] != sk[:-1]) + 1]
    counts = np.diff(np.r_[starts, len(sk)])
    rank_sorted = np.arange(len(sk)) - np.repeat(starts, counts)
    rank = np.empty(len(sk), np.int64)
    rank[order] = rank_sorted
    return rank


def _preprocess_core(src_c, dst_loc_c):
    lo_e = src_c < 5 * NLOC
    nlo = np.bincount(dst_loc_c[lo_e], minlength=NLOC)
    nhi = np.bincount(dst_loc_c[~lo_e], minlength=NLOC)

    L, Hh, B = _design_thresholds(nlo, nhi)
    caps_lo = _split_caps(L, C_LO)   # [C_LO, 32]
    caps_hi = _split_caps(Hh, C_HI)  # [C_HI, 32]

    # assign each dst to (window, lane): most-constrained first, largest
    # feasible lane first
    node_win = np.full(NLOC, -1, np.int64)
    node_slot = np.full(NLOC, -1, np.int64)
    band_cnt = np.zeros(WSPAN, np.int64)
    for u in np.argsort(B, kind="stable"):
        j = B[u]
        while j >= 0 and band_cnt[j] >= NWIN:
            j -= 1
        assert j >= 0, "lane assignment failed"
        node_win[u] = band_cnt[j]
        node_slot[u] = j
        band_cnt[j] += 1

    perm = np.full(NPAD, -1, np.int64)
    perm_pos = node_win * WSPAN + node_slot
    perm[perm_pos] = np.arange(NLOC)

    # per-edge slot assignment: dst at lane j fills its lane's partitions
    # chunk by chunk (chunk-capacity boundaries from caps_*)
    lo_tot = caps_lo.sum(axis=0)
    hi_tot = caps_hi.sum(axis=0)
    assert (nlo <= lo_tot[node_slot]).all() and (nhi <= hi_tot[node_slot]).all()
    cum_lo = np.cumsum(caps_lo, axis=0)      # [C_LO, 32]
    cum_hi = np.cumsum(caps_hi, axis=0)
    start_lo = np.c_[np.zeros(C_LO, np.int64), np.cumsum(caps_lo, axis=1)[:, :-1]]
    start_hi = np.c_[np.zeros(C_HI, np.int64), np.cumsum(caps_hi, axis=1)[:, :-1]]

    e_dst = dst_loc_c
    e_w = node_win[e_dst]
    e_j = node_slot[e_dst]
    key = e_dst * 2 + (~lo_e).astype(np.int64)
    k = _rank_within_group(key)
    # chunk index: first c with k < cum[c, j], computed per class
    c_e = np.zeros(len(k), np.int64)
    p_e = np.zeros(len(k), np.int64)
    for is_lo, cum, start in ((True, cum_lo, start_lo), (False, cum_hi, start_hi)):
        sel = lo_e if is_lo else ~lo_e
        ce = (k[sel, None] >= cum.T[e_j[sel]]).sum(axis=1)
        prev = np.where(
            ce > 0,
            np.take_along_axis(cum.T[e_j[sel]], np.maximum(ce - 1, 0)[:, None], axis=1)[:, 0],
            0,
        )
        c_e[sel] = ce
        p_e[sel] = start.T[e_j[sel], ce] + (k[sel] - prev)
    assert (p_e >= 0).all() and (p_e < 128).all()
    g_, wg_ = e_w // WG, e_w % WG
    ch_e = np.where(
        lo_e,
        g_ * CH_PER_G + wg_ * C_LO + c_e,
        g_ * CH_PER_G + WG * C_LO + wg_ * C_HI + c_e,
    )
    assert (c_e < np.where(lo_e, C_LO, C_HI)).all()
    slot_of_edge = ch_e * 128 + p_e

    filled = np.zeros(SLOTS, bool)
    filled[slot_of_edge] = True
    assert filled[slot_of_edge].all() and len(np.unique(slot_of_edge)) == len(slot_of_edge)
    dead = np.where(filled.reshape(NCH, 128).T, 0.0, NEG_BIG).astype(np.float32)

    # invariant: each edge's partition belongs to its dst's lane in its chunk
    lane_of = np.stack(
        [np.repeat(np.arange(WSPAN), caps_lo[c]) for c in range(C_LO)]
        + [np.repeat(np.arange(WSPAN), caps_hi[c]) for c in range(C_HI)]
    )  # [C_W, 128] in (lo chunks..., hi chunks...) order
    cidx = np.where(lo_e, c_e, C_LO + c_e)
    assert (lane_of[cidx, p_e] == e_j).all()

    # lane one-hots per chunk index: lh[c] is [32 lanes, 128 partitions]
    lanes = np.arange(WSPAN)
    lh = np.zeros((C_W, WSPAN, 128), np.float32)
    l01 = np.zeros((128, C_W, WSPAN), np.float32)
    for c in range(C_LO):
        lane_map = np.repeat(lanes, caps_lo[c])
        lh[c] = (lane_map[None, :] == lanes[:, None])
        l01[:, c, :] = lh[c].T
    for c in range(C_HI):
        lane_map = np.repeat(lanes, caps_hi[c])
        lh[C_LO + c] = (lane_map[None, :] == lanes[:, None])
        l01[:, C_LO + c, :] = lh[C_LO + c].T

    return dict(
        perm=perm,
        perm_pos=perm_pos,
        slot_of_edge=slot_of_edge,
        esrc=src_c,
        e_lo=lo_e.astype(np.int64),
        dead=dead,
        lh=lh,      # [5, 32, 128]
        l01=l01,    # [128, 5, 32]
    )


def _wrap_idx(flat_i64):
    a = flat_i64.astype(np.int16)
    blk = a.reshape(SLOTS // 16, 16).T
    return np.tile(blk, (8, 1)).copy()


def preprocess(src, dst):
    dst_owner = dst // NLOC
    cores = []
    for c in range(NC):
        m = dst_owner == c
        cores.append(_preprocess_core(src[m], dst[m] - c * NLOC))
    perm_pos_all = np.stack([cores[c]["perm_pos"] for c in range(NC)])
    for c in range(NC):
        cc = cores[c]
        owner = cc["esrc"] // NLOC
        src_loc = cc["esrc"] - owner * NLOC
        gid = owner * NPAD + perm_pos_all[owner, src_loc]
        gl = np.zeros(SLOTS, np.int64)
        gl[cc["slot_of_edge"]] = np.where(cc["e_lo"] == 1, gid, gid - LO_ROWS)
        assert gl.max() < 32768 and gl.min() >= 0
        cc["gidx"] = _wrap_idx(gl)
    return cores


# =====================================================================
# device program
# =====================================================================

def build_program():
    key = "main"
    if key in _PROGRAM_CACHE:
        return _PROGRAM_CACHE[key]
    nc = bacc.Bacc(
        "TRN2",
        target_bir_lowering=False,
        debug=False,
        num_devices=NC,
        num_swdge_queues=4,
        dynamic_dma_scratch_size=32768,
    )
    f32 = mybir.dt.float32
    bf16 = mybir.dt.bfloat16
    i16 = mybir.dt.int16

    hperm = nc.dram_tensor("hperm", [NPAD, D], f32, kind="ExternalInput")
    Wc = [nc.dram_tensor(f"Wc{i}", [D, WGT], bf16, kind="ExternalInput") for i in (1, 2)]
    tinv_d = [nc.dram_tensor(f"tinv{i}", [128, 2, 128], bf16, kind="ExternalInput")
              for i in (1, 2)]
    gidx_d = nc.dram_tensor("gidx", [128, SLOTS // 16], i16, kind="ExternalInput")
    dead_d = nc.dram_tensor("deadd", [128, NCH], bf16, kind="ExternalInput")
    lhall_d = nc.dram_tensor("lhall", [WSPAN, C_W, 128], bf16, kind="ExternalInput")
    l01_d = nc.dram_tensor("l01all", [128, C_W, WSPAN], bf16, kind="ExternalInput")
    out_d = nc.dram_tensor("out", [D, NPAD], f32, kind="ExternalOutput")

    table_loc_l = [nc.dram_tensor(f"table_loc{l}", [NPAD, RW], bf16) for l in range(2)]
    table_full_l = [
        nc.dram_tensor(f"table_full{l}", [NC * NPAD, RW], bf16, addr_space="Shared")
        for l in range(2)
    ]
    er_loc_l = [nc.dram_tensor(f"er_loc{l}", [NWIN, WSPAN, H], bf16) for l in range(2)]
    hpre = [
        [nc.dram_tensor(f"hpre{l}_{g}", [WG * WSPAN, D], bf16) for g in range(NGRP)]
        for l in range(2)
    ]

    with tile.TileContext(nc) as tc:
        with (
            tc.tile_pool(name="const", bufs=1) as cpool,
            tc.tile_pool(name="sb", bufs=3) as sb,
            tc.tile_pool(name="gather", bufs=3) as gp,
            tc.tile_pool(name="spool", bufs=3) as spool,
            tc.tile_pool(name="psA", bufs=2, space="PSUM") as psA,
            tc.tile_pool(name="psW", bufs=3, space="PSUM") as psW,
            tc.tile_pool(name="psD", bufs=1, space="PSUM") as psD,
        ):
            ident = cpool.tile([128, 128], f32)
            make_identity(nc, ident[:])
            gix = cpool.tile([128, SLOTS // 16], i16)
            nc.sync.dma_start(out=gix[:], in_=gidx_d[:])
            dead_sb = cpool.tile([128, NCH], bf16)
            nc.sync.dma_start(out=dead_sb[:], in_=dead_d[:])
            lh_sb = cpool.tile([WSPAN, C_W, 128], bf16)
            nc.sync.dma_start(out=lh_sb[:], in_=lhall_d[:])
            l01_sb = cpool.tile([128, C_W, WSPAN], bf16)
            nc.sync.dma_start(out=l01_sb[:], in_=l01_d[:])
            W_sb = []
            tinv_sb = []
            for l in range(2):
                w = cpool.tile([128, 2, WGT], bf16, tag=f"W{l}")
                nc.sync.dma_start(out=w[:], in_=Wc[l][:].rearrange("(k p) n -> p k n", p=128))
                W_sb.append(w)
                tv = cpool.tile([128, 2, 128], bf16, tag=f"Tv{l}")
                nc.sync.dma_start(out=tv[:], in_=tinv_d[l][:])
                tinv_sb.append(tv)

            # ---------------- phase A tails (shared) ----------------
            def phaseA_tail(layer, t, hT):
                pf = psA.tile([128, WGT], f32, tag="pf")
                for k in range(2):
                    nc.tensor.matmul(
                        pf[:], hT[:, k, :], W_sb[layer][:, k, :],
                        start=(k == 0), stop=(k == 1),
                    )
                stg = sb.tile([128, RW], bf16, tag="stgA", bufs=4)
                nc.vector.tensor_copy(out=stg[:], in_=pf[:, 0:D])
                nc.scalar.dma_start(
                    out=table_loc_l[layer][t * 128:(t + 1) * 128, :], in_=stg[:]
                )
                erst = sb.tile([128, H], bf16, tag="erst")
                nc.vector.tensor_copy(out=erst[:], in_=pf[:, D:D + H])
                nc.sync.dma_start(
                    out=er_loc_l[layer][t * 4:(t + 1) * 4, :, :]
                    .rearrange("w j h -> (w j) h"),
                    in_=erst[:],
                )

            def loadA0(t):
                ht = sb.tile([128, D], f32, tag="ht", bufs=4)
                nc.sync.dma_start(out=ht[:], in_=hperm[t * 128:(t + 1) * 128, :])
                return ht

            def phaseA0_tile(t, ht):
                pt = psA.tile([128, 256], f32, tag="pt")
                for k in range(2):
                    nc.tensor.transpose(
                        pt[:, k * 128:(k + 1) * 128], ht[:, k * 128:(k + 1) * 128],
                        ident[:],
                    )
                hT = sb.tile([128, 2, 128], bf16, tag="hT")
                nc.vector.tensor_copy(
                    out=hT[:], in_=pt[:].rearrange("p (k c) -> p k c", k=2)
                )
                phaseA_tail(0, t, hT)

            def loadM(layer, t):
                mt = sb.tile([128, D], bf16, tag="htb", bufs=4)
                nc.sync.dma_start(
                    out=mt[:],
                    in_=hpre[layer][t // 2][(t % 2) * 128:(t % 2) * 128 + 128, :],
                )
                return mt

            def phaseM_tile(layer, t, mt, dnt):
                # recover true features: elu((M / dn) @ Tinv); layer 0 feeds
                # layer-2 table build, layer 1 writes the final output.
                rc = sb.tile([128, H], f32, tag="rc")
                nc.vector.tensor_scalar_max(rc[:], dnt[:], 1e-30)
                nc.vector.reciprocal(rc[:], rc[:])
                mh = sb.tile([128, H, F], f32, tag="mh")
                nc.vector.tensor_mul(
                    out=mh[:],
                    in0=mt[:].rearrange("p (h f) -> p h f", h=H),
                    in1=rc[:].rearrange("p (h o) -> p h o", o=1)
                    .to_broadcast([128, H, F]),
                )
                mh2 = mh[:].rearrange("p h f -> p (h f)")
                pt = psA.tile([128, 256], f32, tag="pt")
                for k in range(2):
                    nc.tensor.transpose(
                        pt[:, k * 128:(k + 1) * 128], mh2[:, k * 128:(k + 1) * 128],
                        ident[:],
                    )
                mhT = sb.tile([128, 2, 128], bf16, tag="mhT")
                nc.vector.tensor_copy(
                    out=mhT[:], in_=pt[:].rearrange("p (k c) -> p k c", k=2)
                )
                pv = psA.tile([128, 2, 128], f32, tag="pt")
                for m in range(2):
                    nc.tensor.matmul(
                        pv[:, m, :], tinv_sb[layer][:, m, :], mhT[:, m, :],
                        start=True, stop=True,
                    )
                r = sb.tile([128, 2, 128], f32, tag="mn")
                nc.scalar.activation(r[:], pv[:], mybir.ActivationFunctionType.Relu)
                mn = sb.tile([128, 2, 128], f32, tag="q")
                nc.vector.tensor_sub(mn[:], pv[:], r[:])
                q = sb.tile([128, 2, 128], f32, tag="q2")
                nc.scalar.activation(q[:], mn[:], mybir.ActivationFunctionType.Exp)
                o = sb.tile([128, 2, 128], f32, tag="o")
                nc.vector.tensor_add(o[:], r[:], q[:])
                if layer == 0:
                    hT = sb.tile([128, 2, 128], bf16, tag="hT2")
                    nc.vector.tensor_scalar_add(hT[:], o[:], -1.0)
                    phaseA_tail(1, t, hT)
                else:
                    of = sb.tile([128, 2, 128], f32, tag="of")
                    nc.vector.tensor_scalar_add(of[:], o[:], -1.0)
                    nc.scalar.dma_start(
                        out=out_d[:, t * 128:(t + 1) * 128]
                        .rearrange("(m d) n -> d m n", m=2),
                        in_=of[:],
                    )

            def load_tiles(g):
                if 1 <= g <= 24:
                    return [2 * (g - 1), 2 * (g - 1) + 1]
                return []

            def compute_tiles(g):
                return [2 * (g - 2), 2 * (g - 2) + 1] if g >= 2 else []

            for layer in range(2):
                # ------- phase A (layer 0 up-front; layer 1 interleaved into B0)
                if layer == 0:
                    for t in range(NTILES):
                        phaseA0_tile(t, loadA0(t))

                nc.gpsimd.collective_compute(
                    "AllGather",
                    mybir.AluOpType.bypass,
                    replica_groups=[list(range(NC))],
                    ins=[table_loc_l[layer].ap().opt()],
                    outs=[table_full_l[layer].ap().opt()],
                )

                # ---------------- phase B: edge aggregation ----------------
                erj_tiles = {}
                dn_tiles = {}

                def load_erj(g):
                    erj = sb.tile([WSPAN, WG, H], bf16, tag="erj")
                    nc.sync.dma_start(
                        out=erj[:],
                        in_=er_loc_l[layer][g * WG:(g + 1) * WG, :, :]
                        .rearrange("w j h -> j w h"),
                    )
                    erj_tiles[g] = erj

                load_erj(0)
                ht_tiles = {}
                g_tiles = {}

                def issue_gathers(g):
                    tf = table_full_l[layer]
                    glo = gp.tile([128, WG * C_LO, RW], bf16, tag="Glo")
                    for c5 in range(C_LO):
                        call = g * CALLS_PER_G + c5
                        nc.gpsimd.dma_gather(
                            out_ap=glo[:, c5 * 8:(c5 + 1) * 8, :],
                            in_ap=tf[0:LO_ROWS, :],
                            idxs_ap=gix[:, call * 64:(call + 1) * 64],
                            num_idxs=1024,
                            num_idxs_reg=1024,
                            elem_size=RW,
                            single_packet=True,
                            queue_num=call % 4,
                        )
                    ghi = gp.tile([128, WG * C_HI, RW], bf16, tag="Ghi")
                    for c2 in range(C_HI):
                        call = g * CALLS_PER_G + C_LO + c2
                        nc.gpsimd.dma_gather(
                            out_ap=ghi[:, c2 * 8:(c2 + 1) * 8, :],
                            in_ap=tf[LO_ROWS:NC * NPAD, :],
                            idxs_ap=gix[:, call * 64:(call + 1) * 64],
                            num_idxs=1024,
                            num_idxs_reg=1024,
                            elem_size=RW,
                            single_packet=True,
                            queue_num=call % 4,
                        )
                    g_tiles[g] = (glo, ghi)

                issue_gathers(0)
                issue_gathers(1)
                for g in range(NGRP):
                    if g + 2 < NGRP:
                        issue_gathers(g + 2)
                    glo, ghi = g_tiles.pop(g)

                    # er per slot via per-chunk lane one-hot matmuls
                    erj = erj_tiles.pop(g)
                    pe_ = psW.tile([128, C_W * WG * H], f32, tag="pw")
                    for c in range(C_W):
                        nc.tensor.matmul(
                            pe_[:, c * WG * H:(c + 1) * WG * H],
                            lh_sb[:, c, :],
                            erj[:].rearrange("j w h -> j (w h)"),
                            start=True, stop=True,
                        )
                    erps = sb.tile([128, C_W, WG, H], bf16, tag="erps")
                    nc.vector.tensor_copy(
                        out=erps[:],
                        in_=pe_[:].rearrange("p (c w h) -> p c w h", c=C_W, w=WG),
                    )

                    # el per slot (column 64h of the gathered R rows), with
                    # the dead-slot NEG_BIG mask folded into the same add
                    elv = sb.tile([128, CH_PER_G, H], bf16, tag="elv")
                    nc.vector.tensor_add(
                        out=elv[:, 0:WG * C_LO, :]
                        .rearrange("p wc (h o) -> p wc h o", o=1),
                        in0=glo[:].rearrange("p wc (h f) -> p wc h f", h=H)
                        [:, :, :, 0:1],
                        in1=dead_sb[:, g * CH_PER_G:g * CH_PER_G + WG * C_LO]
                        .rearrange("p (c o u) -> p c o u", o=1, u=1)
                        .to_broadcast([128, WG * C_LO, H, 1]),
                    )
                    nc.vector.tensor_add(
                        out=elv[:, WG * C_LO:, :]
                        .rearrange("p wc (h o) -> p wc h o", o=1),
                        in0=ghi[:].rearrange("p wc (h f) -> p wc h f", h=H)
                        [:, :, :, 0:1],
                        in1=dead_sb[:, g * CH_PER_G + WG * C_LO:(g + 1) * CH_PER_G]
                        .rearrange("p (c o u) -> p c o u", o=1, u=1)
                        .to_broadcast([128, WG * C_HI, H, 1]),
                    )

                    # compact per-slot attention logits: [128, 40, H]
                    sv = sb.tile([128, CH_PER_G, H], bf16, tag="sv")
                    nc.vector.tensor_add(
                        out=sv[:, 0:WG * C_LO, :].rearrange("p (w c) h -> p w c h", c=C_LO),
                        in0=elv[:, 0:WG * C_LO, :].rearrange("p (w c) h -> p w c h", c=C_LO),
                        in1=erps[:, 0:C_LO].rearrange("p c w h -> p w c h"),
                    )
                    nc.vector.tensor_add(
                        out=sv[:, WG * C_LO:, :].rearrange("p (w c) h -> p w c h", c=C_HI),
                        in0=elv[:, WG * C_LO:, :].rearrange("p (w c) h -> p w c h", c=C_HI),
                        in1=erps[:, C_LO:].rearrange("p c w h -> p w c h"),
                    )
                    nc.scalar.activation(
                        sv[:], sv[:], mybir.ActivationFunctionType.Prelu,
                        alpha=NEG_SLOPE,
                    )
                    nc.scalar.activation(sv[:], sv[:], mybir.ActivationFunctionType.Exp)

                    # softmax denominators, node-major: dg[(wg%4)*32+j, wg//4, h]
                    dg = psD.tile([128, 2, H], f32, tag="dg")
                    for wg in range(WG):
                        chunks = [(c, wg * C_LO + c) for c in range(C_LO)] + [
                            (C_LO + c, WG * C_LO + wg * C_HI + c) for c in range(C_HI)]
                        for i, (cls, ch) in enumerate(chunks):
                            nc.tensor.matmul(
                                dg[(wg % 4) * 32:(wg % 4) * 32 + 32, wg // 4, :],
                                l01_sb[:, cls, :],
                                sv[:, ch, :],
                                start=(i == 0), stop=(i == C_W - 1),
                                tile_position=(0, (wg % 4) * 32),
                            )
                    for half in range(2):
                        dnt = sb.tile([128, H], f32, tag="dnt", bufs=6)
                        nc.vector.tensor_copy(out=dnt[:], in_=dg[:, half, :])
                        dn_tiles[2 * g + half] = dnt

                    # expand to one-hot [128, 40, H, 32] via lane mask multiply
                    # (one multiply per chunk index: ISA allows max 3 free dims)
                    stb = spool.tile([128, CH_PER_G, H, WSPAN], bf16, tag="S")
                    for c in range(C_W):
                        nch_, c0 = (C_LO, 0) if c < C_LO else (C_HI, WG * C_LO)
                        ci = c if c < C_LO else c - C_LO
                        nc.vector.tensor_mul(
                            out=stb[:, c0:c0 + WG * nch_]
                            .rearrange("p (w c) h j -> p c w h j", c=nch_)[:, ci],
                            in0=sv[:, c0:c0 + WG * nch_, :]
                            .rearrange("p (w c) (h o) -> p c w h o", c=nch_, o=1)[:, ci]
                            .to_broadcast([128, WG, H, WSPAN]),
                            in1=l01_sb[:, c, :]
                            .rearrange("p (w h j) -> p w h j", w=1, h=1)
                            .to_broadcast([128, WG, H, WSPAN]),
                        )

                    if g + 1 < NGRP:
                        load_erj(g + 1)
                        for t2 in compute_tiles(g + 1):
                            ht_tiles[t2] = loadM(layer, t2)
                    for t2 in compute_tiles(g):
                        phaseM_tile(layer, t2, ht_tiles.pop(t2), dn_tiles.pop(t2))

                    stg = sb.tile([128, WG, D], bf16, tag="stgB", bufs=2)
                    for wg in range(WG):
                        pw = psW.tile([128, D], f32, tag="pw")
                        chunks = [wg * C_LO + k for k in range(C_LO)] + [
                            WG * C_LO + wg * C_HI + k for k in range(C_HI)]
                        for i, ch in enumerate(chunks):
                            rhs = (
                                glo[:, ch, :]
                                if ch < WG * C_LO
                                else ghi[:, ch - WG * C_LO, :]
                            )
                            nc.tensor.matmul(
                                pw[:],
                                stb[:, ch, :, :].rearrange("p h j -> p (h j)"),
                                rhs,
                                start=(i == 0),
                                stop=(i == C_W - 1),
                            )
                        if wg % 2 == 0:
                            nc.vector.tensor_copy(out=stg[:, wg, :], in_=pw[:])
                        else:
                            nc.scalar.copy(out=stg[:, wg, :], in_=pw[:])
                    dstl = hpre[layer][g]
                    for h in range(H):
                        nc.scalar.dma_start(
                            out=dstl[:, h * F:(h + 1) * F]
                            .rearrange("(w j) f -> j w f", j=WSPAN),
                            in_=stg[h * WSPAN:(h + 1) * WSPAN, :, h * F:h * F + F],
                        )

                for t2 in range(46, 50):
                    if t2 in ht_tiles:
                        ht = ht_tiles.pop(t2)
                    else:
                        ht = loadM(layer, t2)
                    phaseM_tile(layer, t2, ht, dn_tiles.pop(t2))

    nc.compile()
    _PROGRAM_CACHE[key] = nc
    return nc


# =====================================================================
# entry point
# =====================================================================

def _build_T(al):
    """T block-diag per head: block column 0 = al_h (so R col 64h = el_h),
    remaining columns = unit vectors of the other 63 features."""
    al = np.asarray(al, np.float64)
    T = np.zeros((D, D), np.float64)
    for h in range(H):
        k = int(np.argmax(np.abs(al[h])))
        T[64 * h:64 * h + 64, 64 * h] = al[h]
        cols = [f for f in range(F) if f != k]
        for j, f in enumerate(cols, start=1):
            T[64 * h + f, 64 * h + j] = 1.0
    Tinv = np.linalg.inv(T)
    assert np.abs(T @ Tinv - np.eye(D)).max() < 1e-9
    return T, Tinv


def _host_wc(W, al, ar):
    T, Tinv = _build_T(al)
    Wt = W.astype(np.float64) @ T
    Wr = W.reshape(D, H, F)
    wr = np.einsum("dhf,hf->dh", Wr, ar)
    Wc = np.concatenate([Wt, wr], axis=1).astype(np.float32)   # [256, 260]
    tinv = np.zeros((128, 2, 128), np.float32)
    for m in range(2):
        tinv[:, m, :] = Tinv[m * 128:(m + 1) * 128, m * 128:(m + 1) * 128]
    return Wc, tinv


def _to_bf16(x):
    import ml_dtypes
    return np.asarray(x).astype(ml_dtypes.bfloat16)


def build_in_maps(h, src, dst, W1, al1, ar1, W2, al2, ar2):
    cores = preprocess(np.asarray(src), np.asarray(dst))
    Wc1, tinv1 = _host_wc(W1, al1, ar1)
    Wc2, tinv2 = _host_wc(W2, al2, ar2)
    in_maps = []
    for c in range(NC):
        cc = cores[c]
        hp = np.zeros((NPAD, D), np.float32)
        valid = cc["perm"] >= 0
        hp[valid] = h[c * NLOC + cc["perm"][valid]]
        in_maps.append(
            dict(
                hperm=hp,
                Wc1=_to_bf16(Wc1),
                Wc2=_to_bf16(Wc2),
                tinv1=_to_bf16(tinv1),
                tinv2=_to_bf16(tinv2),
                gidx=cc["gidx"],
                deadd=_to_bf16(cc["dead"]),
                lhall=_to_bf16(np.transpose(cc["lh"], (1, 0, 2)).copy()),
                l01all=_to_bf16(cc["l01"]),
            )
        )
    return cores, in_maps


def kernel(h, src, dst, W1, al1, ar1, b1, W2, al2, ar2, b2):
    assert not np.any(b1) and not np.any(b2), "nonzero bias not supported"
    nc = build_program()
    cores, in_maps = build_in_maps(h, src, dst, W1, al1, ar1, W2, al2, ar2)
    res = run_bass_kernel_spmd(nc, in_maps, core_ids=list(range(NC)))
    out = np.empty((N, D), np.float32)
    for c in range(NC):
        o = np.asarray(res.results[c]["out"])     # [D, NPAD] transposed
        pos = cores[c]["perm_pos"]
        out[c * NLOC:(c + 1) * NLOC] = o.T[pos]
    return out


if __name__ == "__main__":
    rng = np.random.default_rng(0)
    h = rng.normal(size=(N, D)).astype(np.float32)
    src = rng.integers(0, N, size=E).astype(np.int32)
    dst = rng.integers(0, N, size=E).astype(np.int32)
    W1 = (rng.normal(size=(D, D)) * 0.05).astype(np.float32)
    al1 = (rng.normal(size=(H, F)) * 0.05).astype(np.float32)
    ar1 = (rng.normal(size=(H, F)) * 0.05).astype(np.float32)
    b1 = np.zeros(D, np.float32)
    out = kernel(h=h, src=src, dst=dst, W1=W1, al1=al1, ar1=ar1, b1=b1,
                 W2=W1, al2=al1, ar2=ar1, b2=b1)
    print("out", out.shape, out[:2, :4])
